# revision 20
# baseline (speedup 1.0000x reference)
"""Chamfer loss (adapted) on 8 TRN2 NeuronCores via Bass/Tile.

Problem: B=2, N=16384, M=8192, D=3
  w = softmax(weights, axis=1)
  dist[b,n,m] = ||p1[b,n] - p2[b,m]||^2  (via sq1 + sq2 - 2*cross)
  loss = mean_b( sum_n w*min_m dist + mean_m min_n dist )

Sharding: core c -> batch b = c//4, quarter q = c%4. Each core computes the
distance matrix ONCE for rows n in its quarter (4096) x all M=8192 columns:
  min1 (row mins)    -> per-core, no communication
  min2 (column mins) -> partial mins over the core's rows, then
                        AllReduce-min across the batch's 4 cores
This halves TensorE work vs computing both (N,M) and (M,N) matrices, and the
bf16 dist tiles produced for min1 are reused for min2.

Numerics: distances need ~1e-4 abs accuracy but the terms are O(10)
(catastrophic cancellation), so the cross term cannot use raw bf16 matmul.
Each coordinate x is split hi/lo (x ~= xh + xl, both bf16); the K=15
augmented contraction computes R[n,m] = -2*sum_d x_d*y_d + ||y||^2 exactly
over the bf16-split points, accumulated in fp32 PSUM (bf16*bf16 products are
exact in fp32). TensorE streams 1 column/cycle regardless of K, so K=15
costs the same as K=5 but keeps full precision. ScalarE converts PSUM->SBUF
bf16 while adding the per-row sq1 bias, so the bf16 rounding happens on the
small biased distance (validated: final rel err ~1e-5).

Engine budget per core (measured rates): ScalarE converts 128 blocks x
1.92us = 246us; VectorE tournament row-mins + running column-mins ~360us;
TensorE 512 matmuls ~220us at the cold 1.2GHz clock (hidden).
"""

import os
import numpy as np
import ml_dtypes

bf16 = ml_dtypes.bfloat16

B, N, M, D = 2, 16384, 8192, 3
NSH = N // 4                       # 4096 query rows per core
K = 15                             # augmented contraction depth
BLK = 2048                         # free-dim columns per PSUM block
NRT, NBLK = NSH // 128, M // BLK   # 32 row-tiles x 4 blocks

# Columns of the min2 running-min handled by GpSimd (rest on VectorE).
# NOTE: walrus rejects InstTensorTensor on the Pool engine for TRN2
# ("Instruction engine check failed (Pool)"), so this stays 0.
GP_COLS = int(os.environ.get("CHAMFER_GP_COLS", "0"))

_compiled = None
_last_results = None


def _build():
    from contextlib import ExitStack
    import concourse.mybir as mybir
    import concourse.tile as tile
    from concourse import bacc
    from concourse.masks import make_identity

    f32, bf = mybir.dt.float32, mybir.dt.bfloat16
    X = mybir.AxisListType.X
    MIN, ADD, MULT = mybir.AluOpType.min, mybir.AluOpType.add, mybir.AluOpType.mult
    IDENT, EXP = mybir.ActivationFunctionType.Identity, mybir.ActivationFunctionType.Exp

    nc = bacc.Bacc("TRN2", target_bir_lowering=False, debug=False, num_devices=8)

    q1 = nc.dram_tensor("q1", (K, NSH), bf, kind="ExternalInput").ap()
    r2 = nc.dram_tensor("r2", (K, M), bf, kind="ExternalInput").ap()
    s1a = nc.dram_tensor("s1a", (128, NRT), f32, kind="ExternalInput").ap()
    wmat = nc.dram_tensor("wmat", (128, 128), f32, kind="ExternalInput").ap()
    wsh = nc.dram_tensor("wsh", (NSH // 128, 128), f32, kind="ExternalInput").ap()
    out = nc.dram_tensor("out", (1, 1), f32, kind="ExternalOutput").ap()
    m2out = nc.dram_tensor("m2out", (128, 64), f32, kind="ExternalOutput").ap()

    with tile.TileContext(nc) as tc, ExitStack() as ctx:
        const = ctx.enter_context(tc.tile_pool(name="const", bufs=1))
        psum = ctx.enter_context(tc.tile_pool(name="psum", bufs=2, space="PSUM"))
        conv = ctx.enter_context(tc.tile_pool(name="conv", bufs=3))
        trn = ctx.enter_context(tc.tile_pool(name="trn", bufs=2))
        coll = ctx.enter_context(tc.tile_pool(name="coll", bufs=2))
        dram = ctx.enter_context(tc.tile_pool(name="dram", bufs=1, space="DRAM"))

        q1t = const.tile([K, NSH], bf, tag="q1t")
        nc.sync.dma_start(q1t[:], q1[:])
        r2t = const.tile([K, M], bf, tag="r2t")
        nc.sync.dma_start(r2t[:], r2[:])
        s1t = const.tile([128, NRT], f32, tag="s1t")
        nc.sync.dma_start(s1t[:], s1a[:])
        wmt = const.tile([128, 128], f32, tag="wmt")
        nc.sync.dma_start(wmt[:], wmat[:])
        wst = const.tile([NSH // 128, 128], f32, tag="wst")
        nc.sync.dma_start(wst[:], wsh[:])

        min1 = const.tile([128, NRT], f32, tag="min1")
        acc = const.tile([128, M], bf, tag="acc")    # running column mins

        # ---- softmax prep (depends only on input DMAs; fills early gaps) ----
        ewm = const.tile([128, 128], f32, tag="ewm")
        nc.scalar.activation(ewm[:], wmt[:], EXP)
        zcol = const.tile([128, 1], f32, tag="zcol")
        nc.vector.tensor_reduce(zcol[:], ewm[:], axis=X, op=ADD)
        ones = const.tile([128, 1], f32, tag="ones")
        nc.gpsimd.memset(ones[:], 1.0)
        wse = const.tile([NSH // 128, 128], f32, tag="wse")
        nc.scalar.activation(wse[:], wst[:], EXP)
        identb = const.tile([128, 128], bf, tag="identb")
        make_identity(nc, identb[:])
        identf = const.tile([32, 32], f32, tag="identf")
        make_identity(nc, identf[:])

        # PE clock warm-up: dense matmuls push the HAM clock gate up before
        # the real stream starts. Results are never read.
        wm = psum.tile([128, BLK], f32, tag="blk")
        for i in range(16):
            nc.tensor.matmul(wm[:, (i % 4) * 512:(i % 4 + 1) * 512],
                             q1t[:, 0:128], r2t[:, 0:512],
                             start=True, stop=True)

        for rt in range(NRT):
            lhsT = q1t[:, rt * 128:(rt + 1) * 128]
            bias_col = s1t[:, rt:rt + 1]
            cvrow = conv.tile([128, M], bf, tag="cvrow")
            for j in range(NBLK):
                ps = psum.tile([128, BLK], f32, tag="blk")
                for k in range(4):
                    nc.tensor.matmul(
                        ps[:, k * 512:(k + 1) * 512], lhsT,
                        r2t[:, (j * 4 + k) * 512:(j * 4 + k + 1) * 512],
                        start=True, stop=True)
                # convert + bias: cv = bf16(R + sq1[row]) = bf16(dist)
                nc.scalar.activation(cvrow[:, j * BLK:(j + 1) * BLK], ps[:],
                                     IDENT, bias=bias_col, scale=1.0)
            # row-min tournament (min1) over the whole row-tile
            t1 = trn.tile([128, M // 2], bf, tag="t1")
            nc.vector.tensor_tensor(
                t1[:], cvrow[:, 0:M // 2], cvrow[:, M // 2:M], op=MIN)
            t2 = trn.tile([128, M // 4], bf, tag="t2")
            nc.vector.tensor_tensor(
                t2[:], t1[:, 0:M // 4], t1[:, M // 4:M // 2], op=MIN)
            t3 = trn.tile([128, M // 8], bf, tag="t3")
            nc.vector.tensor_tensor(
                t3[:], t2[:, 0:M // 8], t2[:, M // 8:M // 4], op=MIN)
            nc.vector.tensor_reduce(
                min1[:, rt:rt + 1], t3[:], axis=X, op=MIN)
            # running column-min (min2)
            if rt == 0:
                nc.vector.tensor_copy(acc[:], cvrow[:])
            else:
                nc.vector.tensor_tensor(acc[:], acc[:], cvrow[:], op=MIN)

        # ---- min2 tail: fold partitions via PE transpose + reduce ----
        min2t = const.tile([128, 64], f32, tag="min2t")
        for g in range(8):                     # 8 groups x 8 col-blocks
            pt = psum.tile([128, BLK], f32, tag="blk")
            ptb = pt[:].bitcast(bf)            # [128, 4096] bf16 view
            for kk in range(8):
                cb = g * 8 + kk
                nc.tensor.transpose(ptb[:, kk * 128:(kk + 1) * 128],
                                    acc[:, cb * 128:(cb + 1) * 128],
                                    identb[:])
            nc.vector.tensor_reduce(
                min2t[:, g * 8:(g + 1) * 8],
                ptb[:, 0:1024].rearrange("p (b f) -> p b f", f=128),
                axis=X, op=MIN)

        # min2 partials go to the host, which does the tiny cross-core
        # elementwise-min + sum (cheaper than a ~20us on-device AllReduce).
        nc.sync.dma_start(m2out[:], min2t[:])

        # ---- weighted sum term1 -> partial scalar ----
        pz = psum.tile([128, BLK], f32, tag="blk")
        # Z = sum_n exp(w[n])  (cross-partition sum via PE)
        nc.tensor.matmul(pz[0:1, 0:1], zcol[:], ones[:], start=True, stop=True)
        # exp(w_shard) transposed into min1's [p, rt] layout
        nc.tensor.transpose(pz[0:128, 512:512 + NSH // 128], wse[:],
                            identf[:])
        ewsh = const.tile([128, NRT], f32, tag="ewsh")
        nc.scalar.copy(ewsh[:], pz[0:128, 512:512 + NRT])

        tmp = const.tile([128, NRT], f32, tag="tmp")
        t1v = const.tile([128, 1], f32, tag="t1v")
        nc.vector.scalar_tensor_tensor(
            tmp[:], ewsh[:], 1.0, min1[:], op0=MULT, op1=MULT,
            accum_out=t1v[:])
        nc.tensor.matmul(pz[0:1, 1024:1025], t1v[:], ones[:],
                         start=True, stop=True)

        fin = const.tile([1, 4], f32, tag="fin")
        nc.scalar.copy(fin[0:1, 0:1], pz[0:1, 0:1])
        nc.scalar.copy(fin[0:1, 1:2], pz[0:1, 1024:1025])
        zr = const.tile([1, 1], f32, tag="zr")
        nc.vector.reciprocal(zr[:], fin[0:1, 0:1])
        osc = const.tile([1, 1], f32, tag="osc")
        nc.vector.tensor_mul(osc[:], fin[0:1, 1:2], zr[:])
        nc.sync.dma_start(out[:], osc[:])

    nc.compile()
    return nc


def _split(v):
    h = v.astype(bf16)
    l = (v - h.astype(np.float32)).astype(bf16)
    return h, l


def _query_aug(P):
    """P [n,3] f32 -> [15, n] bf16 (lhsT / stationary side)."""
    rows = []
    for dd in range(3):
        h, l = _split(P[:, dd])
        rows += [h, h, l, l]
    one = np.ones(P.shape[0], dtype=bf16)
    rows += [one, one, one]
    return np.stack(rows, 0)


def _ref_aug(Q):
    """Q [m,3] f32 -> [15, m] bf16 (rhs / moving side, carries -2y and sq)."""
    rows = []
    eff = np.zeros(Q.shape, np.float64)
    for dd in range(3):
        h, l = _split(Q[:, dd])
        h2 = (-2.0 * h.astype(np.float32)).astype(bf16)
        l2 = (-2.0 * l.astype(np.float32)).astype(bf16)
        rows += [h2, l2, h2, l2]
        eff[:, dd] = h.astype(np.float64) + l.astype(np.float64)
    sq = (eff ** 2).sum(-1).astype(np.float32)
    s0 = sq.astype(bf16)
    r = sq - s0.astype(np.float32)
    s1 = r.astype(bf16)
    s2 = (r - s1.astype(np.float32)).astype(bf16)
    rows += [s0, s1, s2]
    return np.stack(rows, 0)


def _sq_eff(P):
    eff = np.zeros(P.shape, np.float64)
    for dd in range(3):
        h, l = _split(P[:, dd])
        eff[:, dd] = h.astype(np.float64) + l.astype(np.float64)
    return (eff ** 2).sum(-1).astype(np.float32)


def kernel(points1, points2, weights):
    global _compiled, _last_results
    from concourse.bass_utils import run_bass_kernel_spmd

    p1 = np.ascontiguousarray(np.asarray(points1, dtype=np.float32))
    p2 = np.ascontiguousarray(np.asarray(points2, dtype=np.float32))
    w = np.ascontiguousarray(np.asarray(weights, dtype=np.float32))

    if _compiled is None:
        _compiled = _build()

    in_maps = []
    for c in range(8):
        b, q = divmod(c, 4)
        p1b, p2b, wb = p1[b], p2[b], w[b]
        n0 = q * NSH
        sq1 = _sq_eff(p1b[n0:n0 + NSH])
        in_maps.append({
            "q1": np.ascontiguousarray(_query_aug(p1b[n0:n0 + NSH])),
            "r2": np.ascontiguousarray(_ref_aug(p2b)),
            "s1a": np.ascontiguousarray(sq1.reshape(NRT, 128).T),
            "wmat": np.ascontiguousarray(wb.reshape(128, 128)),
            "wsh": np.ascontiguousarray(wb[n0:n0 + NSH].reshape(NSH // 128, 128)),
        })

    trace = os.environ.get("CHAMFER_TRACE", "0") == "1"
    res = run_bass_kernel_spmd(_compiled, in_maps, core_ids=list(range(8)),
                               trace=trace)
    _last_results = res
    total = 0.0
    for b in range(B):
        term1 = sum(float(res.results[4 * b + q]["out"][0, 0]) for q in range(4))
        m2 = np.min([res.results[4 * b + q]["m2out"] for q in range(4)], axis=0)
        total += term1 + float(m2.sum(dtype=np.float64)) / M
    return np.asarray(np.float32(total / B))


# revision 22
# speedup vs baseline: 1.1950x; 1.1950x over previous
"""Chamfer loss (adapted) on 8 TRN2 NeuronCores via Bass/Tile.

Problem: B=2, N=16384, M=8192, D=3
  w = softmax(weights, axis=1)
  dist[b,n,m] = ||p1[b,n] - p2[b,m]||^2  (via sq1 + sq2 - 2*cross)
  loss = mean_b( sum_n w*min_m dist + mean_m min_n dist )

Sharding: core c -> batch b = c//4, quarter q = c%4. Each core computes the
distance matrix ONCE for rows n in its quarter (4096) x all M=8192 columns:
  min1 (row mins)    -> per-core, no communication
  min2 (column mins) -> partial mins over the core's rows, then
                        AllReduce-min across the batch's 4 cores
This halves TensorE work vs computing both (N,M) and (M,N) matrices, and the
bf16 dist tiles produced for min1 are reused for min2.

Numerics: distances need ~1e-4 abs accuracy but the terms are O(10)
(catastrophic cancellation), so the cross term cannot use raw bf16 matmul.
Each coordinate x is split hi/lo (x ~= xh + xl, both bf16); the K=15
augmented contraction computes R[n,m] = -2*sum_d x_d*y_d + ||y||^2 exactly
over the bf16-split points, accumulated in fp32 PSUM (bf16*bf16 products are
exact in fp32). TensorE streams 1 column/cycle regardless of K, so K=15
costs the same as K=5 but keeps full precision. ScalarE converts PSUM->SBUF
bf16 while adding the per-row sq1 bias, so the bf16 rounding happens on the
small biased distance (validated: final rel err ~1e-5).

Engine budget per core (measured rates): ScalarE converts 128 blocks x
1.92us = 246us; VectorE tournament row-mins + running column-mins ~360us;
TensorE 512 matmuls ~220us at the cold 1.2GHz clock (hidden).
"""

import os
import numpy as np
import ml_dtypes

bf16 = ml_dtypes.bfloat16

B, N, M, D = 2, 16384, 8192, 3
NSH = N // 4                       # 4096 query rows per core
K = 15                             # augmented contraction depth
BLK = 2048                         # free-dim columns per PSUM block
NRT, NBLK = NSH // 128, M // BLK   # 32 row-tiles x 4 blocks

# Columns of the min2 running-min handled by GpSimd (rest on VectorE).
# NOTE: walrus rejects InstTensorTensor on the Pool engine for TRN2
# ("Instruction engine check failed (Pool)"), so this stays 0.
GP_COLS = int(os.environ.get("CHAMFER_GP_COLS", "0"))

_compiled = None
_last_results = None


def _build():
    from contextlib import ExitStack
    import concourse.mybir as mybir
    import concourse.tile as tile
    from concourse import bacc
    from concourse.masks import make_identity

    f32, bf = mybir.dt.float32, mybir.dt.bfloat16
    X = mybir.AxisListType.X
    MIN, ADD, MULT = mybir.AluOpType.min, mybir.AluOpType.add, mybir.AluOpType.mult
    IDENT, EXP = mybir.ActivationFunctionType.Identity, mybir.ActivationFunctionType.Exp

    nc = bacc.Bacc("TRN2", target_bir_lowering=False, debug=False, num_devices=8)

    q1 = nc.dram_tensor("q1", (K, NSH), bf, kind="ExternalInput").ap()
    r2 = nc.dram_tensor("r2", (K, M), bf, kind="ExternalInput").ap()
    s1a = nc.dram_tensor("s1a", (128, NRT), f32, kind="ExternalInput").ap()
    wmat = nc.dram_tensor("wmat", (128, 128), f32, kind="ExternalInput").ap()
    wsh = nc.dram_tensor("wsh", (NSH // 128, 128), f32, kind="ExternalInput").ap()
    out = nc.dram_tensor("out", (1, 1), f32, kind="ExternalOutput").ap()
    m2out = nc.dram_tensor("m2out", (128, 64), f32, kind="ExternalOutput").ap()

    with tile.TileContext(nc) as tc, ExitStack() as ctx:
        const = ctx.enter_context(tc.tile_pool(name="const", bufs=1))
        psum = ctx.enter_context(tc.tile_pool(name="psum", bufs=2, space="PSUM"))
        conv = ctx.enter_context(tc.tile_pool(name="conv", bufs=4))
        trn = ctx.enter_context(tc.tile_pool(name="trn", bufs=2))
        coll = ctx.enter_context(tc.tile_pool(name="coll", bufs=2))
        dram = ctx.enter_context(tc.tile_pool(name="dram", bufs=1, space="DRAM"))

        q1t = const.tile([K, NSH], bf, tag="q1t")
        nc.sync.dma_start(q1t[:], q1[:])
        r2t = const.tile([K, M], bf, tag="r2t")
        nc.sync.dma_start(r2t[:], r2[:])
        s1t = const.tile([128, NRT], f32, tag="s1t")
        nc.sync.dma_start(s1t[:], s1a[:])
        wmt = const.tile([128, 128], f32, tag="wmt")
        nc.sync.dma_start(wmt[:], wmat[:])
        wst = const.tile([NSH // 128, 128], f32, tag="wst")
        nc.sync.dma_start(wst[:], wsh[:])

        min1 = const.tile([128, NRT], f32, tag="min1")
        acc = const.tile([128, M], bf, tag="acc")    # running column mins

        # ---- softmax prep (depends only on input DMAs; fills early gaps) ----
        ewm = const.tile([128, 128], f32, tag="ewm")
        nc.scalar.activation(ewm[:], wmt[:], EXP)
        zcol = const.tile([128, 1], f32, tag="zcol")
        nc.vector.tensor_reduce(zcol[:], ewm[:], axis=X, op=ADD)
        ones = const.tile([128, 1], f32, tag="ones")
        nc.gpsimd.memset(ones[:], 1.0)
        wse = const.tile([NSH // 128, 128], f32, tag="wse")
        nc.scalar.activation(wse[:], wst[:], EXP)
        identb = const.tile([128, 128], bf, tag="identb")
        make_identity(nc, identb[:])
        identf = const.tile([32, 32], f32, tag="identf")
        make_identity(nc, identf[:])

        # PE clock warm-up: dense matmuls push the HAM clock gate up before
        # the real stream starts. Results are never read.
        wm = psum.tile([128, BLK], f32, tag="blk")
        for i in range(16):
            nc.tensor.matmul(wm[:, (i % 4) * 512:(i % 4 + 1) * 512],
                             q1t[:, 0:128], r2t[:, 0:512],
                             start=True, stop=True)

        for rt in range(NRT):
            lhsT = q1t[:, rt * 128:(rt + 1) * 128]
            bias_col = s1t[:, rt:rt + 1]
            cvrow = conv.tile([128, M], bf, tag="cvrow")
            for j in range(NBLK):
                ps = psum.tile([128, BLK], f32, tag="blk")
                for k in range(4):
                    nc.tensor.matmul(
                        ps[:, k * 512:(k + 1) * 512], lhsT,
                        r2t[:, (j * 4 + k) * 512:(j * 4 + k + 1) * 512],
                        start=True, stop=True)
                # convert + bias: cv = bf16(R + sq1[row]) = bf16(dist)
                nc.scalar.activation(cvrow[:, j * BLK:(j + 1) * BLK], ps[:],
                                     IDENT, bias=bias_col, scale=1.0)
            # running column-min (min2) first, then tournament L1 — these two
            # are the only cvrow readers; keeping them early releases the
            # cvrow slot quickly so ScalarE converts (and thus PSUM/PE) don't
            # stall on pool back-pressure.
            if rt == 0:
                nc.vector.tensor_copy(acc[:], cvrow[:])
            else:
                nc.vector.tensor_tensor(acc[:], acc[:], cvrow[:], op=MIN)
            # row-min tournament (min1) over the whole row-tile
            t1 = trn.tile([128, M // 2], bf, tag="t1")
            nc.vector.tensor_tensor(
                t1[:], cvrow[:, 0:M // 2], cvrow[:, M // 2:M], op=MIN)
            t2 = trn.tile([128, M // 4], bf, tag="t2")
            nc.vector.tensor_tensor(
                t2[:], t1[:, 0:M // 4], t1[:, M // 4:M // 2], op=MIN)
            t3 = trn.tile([128, M // 8], bf, tag="t3")
            nc.vector.tensor_tensor(
                t3[:], t2[:, 0:M // 8], t2[:, M // 8:M // 4], op=MIN)
            nc.vector.tensor_reduce(
                min1[:, rt:rt + 1], t3[:], axis=X, op=MIN)

        # ---- min2 tail: fold partitions via PE transpose + reduce ----
        min2t = const.tile([128, 64], f32, tag="min2t")
        for g in range(8):                     # 8 groups x 8 col-blocks
            pt = psum.tile([128, BLK], f32, tag="blk")
            ptb = pt[:].bitcast(bf)            # [128, 4096] bf16 view
            for kk in range(8):
                cb = g * 8 + kk
                nc.tensor.transpose(ptb[:, kk * 128:(kk + 1) * 128],
                                    acc[:, cb * 128:(cb + 1) * 128],
                                    identb[:])
            nc.vector.tensor_reduce(
                min2t[:, g * 8:(g + 1) * 8],
                ptb[:, 0:1024].rearrange("p (b f) -> p b f", f=128),
                axis=X, op=MIN)

        # min2 partials go to the host, which does the tiny cross-core
        # elementwise-min + sum (cheaper than a ~20us on-device AllReduce).
        nc.sync.dma_start(m2out[:], min2t[:])

        # ---- weighted sum term1 -> partial scalar ----
        pz = psum.tile([128, BLK], f32, tag="blk")
        # Z = sum_n exp(w[n])  (cross-partition sum via PE)
        nc.tensor.matmul(pz[0:1, 0:1], zcol[:], ones[:], start=True, stop=True)
        # exp(w_shard) transposed into min1's [p, rt] layout
        nc.tensor.transpose(pz[0:128, 512:512 + NSH // 128], wse[:],
                            identf[:])
        ewsh = const.tile([128, NRT], f32, tag="ewsh")
        nc.scalar.copy(ewsh[:], pz[0:128, 512:512 + NRT])

        tmp = const.tile([128, NRT], f32, tag="tmp")
        t1v = const.tile([128, 1], f32, tag="t1v")
        nc.vector.scalar_tensor_tensor(
            tmp[:], ewsh[:], 1.0, min1[:], op0=MULT, op1=MULT,
            accum_out=t1v[:])
        nc.tensor.matmul(pz[0:1, 1024:1025], t1v[:], ones[:],
                         start=True, stop=True)

        fin = const.tile([1, 4], f32, tag="fin")
        nc.scalar.copy(fin[0:1, 0:1], pz[0:1, 0:1])
        nc.scalar.copy(fin[0:1, 1:2], pz[0:1, 1024:1025])
        zr = const.tile([1, 1], f32, tag="zr")
        nc.vector.reciprocal(zr[:], fin[0:1, 0:1])
        osc = const.tile([1, 1], f32, tag="osc")
        nc.vector.tensor_mul(osc[:], fin[0:1, 1:2], zr[:])
        nc.sync.dma_start(out[:], osc[:])

    nc.compile()
    return nc


def _split(v):
    h = v.astype(bf16)
    l = (v - h.astype(np.float32)).astype(bf16)
    return h, l


def _query_aug(P):
    """P [n,3] f32 -> [15, n] bf16 (lhsT / stationary side)."""
    rows = []
    for dd in range(3):
        h, l = _split(P[:, dd])
        rows += [h, h, l, l]
    one = np.ones(P.shape[0], dtype=bf16)
    rows += [one, one, one]
    return np.stack(rows, 0)


def _ref_aug(Q):
    """Q [m,3] f32 -> [15, m] bf16 (rhs / moving side, carries -2y and sq)."""
    rows = []
    eff = np.zeros(Q.shape, np.float64)
    for dd in range(3):
        h, l = _split(Q[:, dd])
        h2 = (-2.0 * h.astype(np.float32)).astype(bf16)
        l2 = (-2.0 * l.astype(np.float32)).astype(bf16)
        rows += [h2, l2, h2, l2]
        eff[:, dd] = h.astype(np.float64) + l.astype(np.float64)
    sq = (eff ** 2).sum(-1).astype(np.float32)
    s0 = sq.astype(bf16)
    r = sq - s0.astype(np.float32)
    s1 = r.astype(bf16)
    s2 = (r - s1.astype(np.float32)).astype(bf16)
    rows += [s0, s1, s2]
    return np.stack(rows, 0)


def _sq_eff(P):
    eff = np.zeros(P.shape, np.float64)
    for dd in range(3):
        h, l = _split(P[:, dd])
        eff[:, dd] = h.astype(np.float64) + l.astype(np.float64)
    return (eff ** 2).sum(-1).astype(np.float32)


def kernel(points1, points2, weights):
    global _compiled, _last_results
    from concourse.bass_utils import run_bass_kernel_spmd

    p1 = np.ascontiguousarray(np.asarray(points1, dtype=np.float32))
    p2 = np.ascontiguousarray(np.asarray(points2, dtype=np.float32))
    w = np.ascontiguousarray(np.asarray(weights, dtype=np.float32))

    if _compiled is None:
        _compiled = _build()

    in_maps = []
    for c in range(8):
        b, q = divmod(c, 4)
        p1b, p2b, wb = p1[b], p2[b], w[b]
        n0 = q * NSH
        sq1 = _sq_eff(p1b[n0:n0 + NSH])
        in_maps.append({
            "q1": np.ascontiguousarray(_query_aug(p1b[n0:n0 + NSH])),
            "r2": np.ascontiguousarray(_ref_aug(p2b)),
            "s1a": np.ascontiguousarray(sq1.reshape(NRT, 128).T),
            "wmat": np.ascontiguousarray(wb.reshape(128, 128)),
            "wsh": np.ascontiguousarray(wb[n0:n0 + NSH].reshape(NSH // 128, 128)),
        })

    trace = os.environ.get("CHAMFER_TRACE", "0") == "1"
    res = run_bass_kernel_spmd(_compiled, in_maps, core_ids=list(range(8)),
                               trace=trace)
    _last_results = res
    total = 0.0
    for b in range(B):
        term1 = sum(float(res.results[4 * b + q]["out"][0, 0]) for q in range(4))
        m2 = np.min([res.results[4 * b + q]["m2out"] for q in range(4)], axis=0)
        total += term1 + float(m2.sum(dtype=np.float64)) / M
    return np.asarray(np.float32(total / B))


# revision 23
# speedup vs baseline: 1.2244x; 1.0246x over previous
"""Chamfer loss (adapted) on 8 TRN2 NeuronCores via Bass/Tile.

Problem: B=2, N=16384, M=8192, D=3
  w = softmax(weights, axis=1)
  dist[b,n,m] = ||p1[b,n] - p2[b,m]||^2  (via sq1 + sq2 - 2*cross)
  loss = mean_b( sum_n w*min_m dist + mean_m min_n dist )

Sharding: core c -> batch b = c//4, quarter q = c%4. Each core computes the
distance matrix ONCE for rows n in its quarter (4096) x all M=8192 columns:
  min1 (row mins)    -> per-core, no communication
  min2 (column mins) -> partial mins over the core's rows, then
                        AllReduce-min across the batch's 4 cores
This halves TensorE work vs computing both (N,M) and (M,N) matrices, and the
bf16 dist tiles produced for min1 are reused for min2.

Numerics: distances need ~1e-4 abs accuracy but the terms are O(10)
(catastrophic cancellation), so the cross term cannot use raw bf16 matmul.
Each coordinate x is split hi/lo (x ~= xh + xl, both bf16); the K=15
augmented contraction computes R[n,m] = -2*sum_d x_d*y_d + ||y||^2 exactly
over the bf16-split points, accumulated in fp32 PSUM (bf16*bf16 products are
exact in fp32). TensorE streams 1 column/cycle regardless of K, so K=15
costs the same as K=5 but keeps full precision. ScalarE converts PSUM->SBUF
bf16 while adding the per-row sq1 bias, so the bf16 rounding happens on the
small biased distance (validated: final rel err ~1e-5).

Engine budget per core (measured rates): ScalarE converts 128 blocks x
1.92us = 246us; VectorE tournament row-mins + running column-mins ~360us;
TensorE 512 matmuls ~220us at the cold 1.2GHz clock (hidden).
"""

import os
import numpy as np
import ml_dtypes

bf16 = ml_dtypes.bfloat16

B, N, M, D = 2, 16384, 8192, 3
NSH = N // 4                       # 4096 query rows per core
K = 15                             # augmented contraction depth
BLK = 2048                         # free-dim columns per PSUM block
NRT, NBLK = NSH // 128, M // BLK   # 32 row-tiles x 4 blocks

# Columns of the min2 running-min handled by GpSimd (rest on VectorE).
# NOTE: walrus rejects InstTensorTensor on the Pool engine for TRN2
# ("Instruction engine check failed (Pool)"), so this stays 0.
GP_COLS = int(os.environ.get("CHAMFER_GP_COLS", "0"))

_compiled = None
_last_results = None


def _build():
    from contextlib import ExitStack
    import concourse.mybir as mybir
    import concourse.tile as tile
    from concourse import bacc
    from concourse.masks import make_identity

    f32, bf = mybir.dt.float32, mybir.dt.bfloat16
    X = mybir.AxisListType.X
    MIN, ADD, MULT = mybir.AluOpType.min, mybir.AluOpType.add, mybir.AluOpType.mult
    IDENT, EXP = mybir.ActivationFunctionType.Identity, mybir.ActivationFunctionType.Exp

    nc = bacc.Bacc("TRN2", target_bir_lowering=False, debug=False, num_devices=8)

    q1 = nc.dram_tensor("q1", (K, NSH), bf, kind="ExternalInput").ap()
    r2 = nc.dram_tensor("r2", (K, M), bf, kind="ExternalInput").ap()
    s1a = nc.dram_tensor("s1a", (128, NRT), f32, kind="ExternalInput").ap()
    wmat = nc.dram_tensor("wmat", (128, 128), f32, kind="ExternalInput").ap()
    wsh = nc.dram_tensor("wsh", (NSH // 128, 128), f32, kind="ExternalInput").ap()
    out = nc.dram_tensor("out", (1, 1), f32, kind="ExternalOutput").ap()
    m2out = nc.dram_tensor("m2out", (128, 64), f32, kind="ExternalOutput").ap()

    with tile.TileContext(nc) as tc, ExitStack() as ctx:
        const = ctx.enter_context(tc.tile_pool(name="const", bufs=1))
        psum = ctx.enter_context(tc.tile_pool(name="psum", bufs=2, space="PSUM"))
        conv = ctx.enter_context(tc.tile_pool(name="conv", bufs=4))
        trn = ctx.enter_context(tc.tile_pool(name="trn", bufs=2))
        coll = ctx.enter_context(tc.tile_pool(name="coll", bufs=2))
        dram = ctx.enter_context(tc.tile_pool(name="dram", bufs=1, space="DRAM"))

        q1t = const.tile([K, NSH], bf, tag="q1t")
        nc.sync.dma_start(q1t[:], q1[:])
        r2t = const.tile([K, M], bf, tag="r2t")
        nc.sync.dma_start(r2t[:], r2[:])
        s1t = const.tile([128, NRT], f32, tag="s1t")
        nc.sync.dma_start(s1t[:], s1a[:])
        wmt = const.tile([128, 128], f32, tag="wmt")
        nc.sync.dma_start(wmt[:], wmat[:])
        wst = const.tile([NSH // 128, 128], f32, tag="wst")
        nc.sync.dma_start(wst[:], wsh[:])

        min1 = const.tile([128, NRT], f32, tag="min1")
        acc = const.tile([128, M], bf, tag="acc")    # running column mins

        # ---- softmax prep (depends only on input DMAs; fills early gaps) ----
        ewm = const.tile([128, 128], f32, tag="ewm")
        nc.scalar.activation(ewm[:], wmt[:], EXP)
        zcol = const.tile([128, 1], f32, tag="zcol")
        nc.vector.tensor_reduce(zcol[:], ewm[:], axis=X, op=ADD)
        ones = const.tile([128, 1], f32, tag="ones")
        nc.gpsimd.memset(ones[:], 1.0)
        wse = const.tile([NSH // 128, 128], f32, tag="wse")
        nc.scalar.activation(wse[:], wst[:], EXP)
        identb = const.tile([128, 128], bf, tag="identb")
        make_identity(nc, identb[:])
        identf = const.tile([32, 32], f32, tag="identf")
        make_identity(nc, identf[:])

        # PE clock warm-up: dense matmuls push the HAM clock gate up before
        # the real stream starts. Results are never read.
        wm = psum.tile([128, BLK], f32, tag="blk")
        for i in range(16):
            nc.tensor.matmul(wm[:, (i % 4) * 512:(i % 4 + 1) * 512],
                             q1t[:, 0:128], r2t[:, 0:512],
                             start=True, stop=True)

        for rt in range(NRT):
            lhsT = q1t[:, rt * 128:(rt + 1) * 128]
            bias_col = s1t[:, rt:rt + 1]
            cvrow = conv.tile([128, M], bf, tag="cvrow")
            for j in range(NBLK):
                ps = psum.tile([128, BLK], f32, tag="blk")
                for k in range(4):
                    nc.tensor.matmul(
                        ps[:, k * 512:(k + 1) * 512], lhsT,
                        r2t[:, (j * 4 + k) * 512:(j * 4 + k + 1) * 512],
                        start=True, stop=True)
                # convert + bias: cv = bf16(R + sq1[row]) = bf16(dist)
                nc.scalar.activation(cvrow[:, j * BLK:(j + 1) * BLK], ps[:],
                                     IDENT, bias=bias_col, scale=1.0)
            # running column-min (min2) first, then tournament L1 — these two
            # are the only cvrow readers; keeping them early releases the
            # cvrow slot quickly so ScalarE converts (and thus PSUM/PE) don't
            # stall on pool back-pressure.
            if rt == 0:
                nc.vector.tensor_copy(acc[:], cvrow[:])
            else:
                nc.vector.tensor_tensor(acc[:], acc[:], cvrow[:], op=MIN)
            # row-min tournament (min1) over the whole row-tile
            t1 = trn.tile([128, M // 2], bf, tag="t1")
            nc.vector.tensor_tensor(
                t1[:], cvrow[:, 0:M // 2], cvrow[:, M // 2:M], op=MIN)
            t2 = trn.tile([128, M // 4], bf, tag="t2")
            nc.vector.tensor_tensor(
                t2[:], t1[:, 0:M // 4], t1[:, M // 4:M // 2], op=MIN)
            t3 = trn.tile([128, M // 8], bf, tag="t3")
            nc.vector.tensor_tensor(
                t3[:], t2[:, 0:M // 8], t2[:, M // 8:M // 4], op=MIN)
            t4 = trn.tile([128, M // 16], bf, tag="t4")
            nc.vector.tensor_tensor(
                t4[:], t3[:, 0:M // 16], t3[:, M // 16:M // 8], op=MIN)
            t5 = trn.tile([128, M // 32], bf, tag="t5")
            nc.vector.tensor_tensor(
                t5[:], t4[:, 0:M // 32], t4[:, M // 32:M // 16], op=MIN)
            nc.vector.tensor_reduce(
                min1[:, rt:rt + 1], t5[:], axis=X, op=MIN)

        # ---- min2 tail: fold partitions via PE transpose + reduce ----
        min2t = const.tile([128, 64], f32, tag="min2t")
        for g in range(8):                     # 8 groups x 8 col-blocks
            pt = psum.tile([128, BLK], f32, tag="blk")
            ptb = pt[:].bitcast(bf)            # [128, 4096] bf16 view
            for kk in range(8):
                cb = g * 8 + kk
                nc.tensor.transpose(ptb[:, kk * 128:(kk + 1) * 128],
                                    acc[:, cb * 128:(cb + 1) * 128],
                                    identb[:])
            nc.vector.tensor_reduce(
                min2t[:, g * 8:(g + 1) * 8],
                ptb[:, 0:1024].rearrange("p (b f) -> p b f", f=128),
                axis=X, op=MIN)

        # min2 partials go to the host, which does the tiny cross-core
        # elementwise-min + sum (cheaper than a ~20us on-device AllReduce).
        nc.sync.dma_start(m2out[:], min2t[:])

        # ---- weighted sum term1 -> partial scalar ----
        pz = psum.tile([128, BLK], f32, tag="blk")
        # Z = sum_n exp(w[n])  (cross-partition sum via PE)
        nc.tensor.matmul(pz[0:1, 0:1], zcol[:], ones[:], start=True, stop=True)
        # exp(w_shard) transposed into min1's [p, rt] layout
        nc.tensor.transpose(pz[0:128, 512:512 + NSH // 128], wse[:],
                            identf[:])
        ewsh = const.tile([128, NRT], f32, tag="ewsh")
        nc.scalar.copy(ewsh[:], pz[0:128, 512:512 + NRT])

        tmp = const.tile([128, NRT], f32, tag="tmp")
        t1v = const.tile([128, 1], f32, tag="t1v")
        nc.vector.scalar_tensor_tensor(
            tmp[:], ewsh[:], 1.0, min1[:], op0=MULT, op1=MULT,
            accum_out=t1v[:])
        nc.tensor.matmul(pz[0:1, 1024:1025], t1v[:], ones[:],
                         start=True, stop=True)

        fin = const.tile([1, 4], f32, tag="fin")
        nc.scalar.copy(fin[0:1, 0:1], pz[0:1, 0:1])
        nc.scalar.copy(fin[0:1, 1:2], pz[0:1, 1024:1025])
        zr = const.tile([1, 1], f32, tag="zr")
        nc.vector.reciprocal(zr[:], fin[0:1, 0:1])
        osc = const.tile([1, 1], f32, tag="osc")
        nc.vector.tensor_mul(osc[:], fin[0:1, 1:2], zr[:])
        nc.sync.dma_start(out[:], osc[:])

    nc.compile()
    return nc


def _split(v):
    h = v.astype(bf16)
    l = (v - h.astype(np.float32)).astype(bf16)
    return h, l


def _query_aug(P):
    """P [n,3] f32 -> [15, n] bf16 (lhsT / stationary side)."""
    rows = []
    for dd in range(3):
        h, l = _split(P[:, dd])
        rows += [h, h, l, l]
    one = np.ones(P.shape[0], dtype=bf16)
    rows += [one, one, one]
    return np.stack(rows, 0)


def _ref_aug(Q):
    """Q [m,3] f32 -> [15, m] bf16 (rhs / moving side, carries -2y and sq)."""
    rows = []
    eff = np.zeros(Q.shape, np.float64)
    for dd in range(3):
        h, l = _split(Q[:, dd])
        h2 = (-2.0 * h.astype(np.float32)).astype(bf16)
        l2 = (-2.0 * l.astype(np.float32)).astype(bf16)
        rows += [h2, l2, h2, l2]
        eff[:, dd] = h.astype(np.float64) + l.astype(np.float64)
    sq = (eff ** 2).sum(-1).astype(np.float32)
    s0 = sq.astype(bf16)
    r = sq - s0.astype(np.float32)
    s1 = r.astype(bf16)
    s2 = (r - s1.astype(np.float32)).astype(bf16)
    rows += [s0, s1, s2]
    return np.stack(rows, 0)


def _sq_eff(P):
    eff = np.zeros(P.shape, np.float64)
    for dd in range(3):
        h, l = _split(P[:, dd])
        eff[:, dd] = h.astype(np.float64) + l.astype(np.float64)
    return (eff ** 2).sum(-1).astype(np.float32)


def kernel(points1, points2, weights):
    global _compiled, _last_results
    from concourse.bass_utils import run_bass_kernel_spmd

    p1 = np.ascontiguousarray(np.asarray(points1, dtype=np.float32))
    p2 = np.ascontiguousarray(np.asarray(points2, dtype=np.float32))
    w = np.ascontiguousarray(np.asarray(weights, dtype=np.float32))

    if _compiled is None:
        _compiled = _build()

    in_maps = []
    for c in range(8):
        b, q = divmod(c, 4)
        p1b, p2b, wb = p1[b], p2[b], w[b]
        n0 = q * NSH
        sq1 = _sq_eff(p1b[n0:n0 + NSH])
        in_maps.append({
            "q1": np.ascontiguousarray(_query_aug(p1b[n0:n0 + NSH])),
            "r2": np.ascontiguousarray(_ref_aug(p2b)),
            "s1a": np.ascontiguousarray(sq1.reshape(NRT, 128).T),
            "wmat": np.ascontiguousarray(wb.reshape(128, 128)),
            "wsh": np.ascontiguousarray(wb[n0:n0 + NSH].reshape(NSH // 128, 128)),
        })

    trace = os.environ.get("CHAMFER_TRACE", "0") == "1"
    res = run_bass_kernel_spmd(_compiled, in_maps, core_ids=list(range(8)),
                               trace=trace)
    _last_results = res
    total = 0.0
    for b in range(B):
        term1 = sum(float(res.results[4 * b + q]["out"][0, 0]) for q in range(4))
        m2 = np.min([res.results[4 * b + q]["m2out"] for q in range(4)], axis=0)
        total += term1 + float(m2.sum(dtype=np.float64)) / M
    return np.asarray(np.float32(total / B))
